# revision 1
# baseline (speedup 1.0000x reference)
"""CRF Viterbi decode (nn_CrfDecodeLayer) Trainium2 Bass kernel.

Problem: B=256, T=512, K=256 tags. Forward max-plus scan over T with
transition matrix trans[K,K], then backtrack to recover argmax tag path.
Output: tags [B, 514] int32 (padded to max_sequence_length + 2).

Sharding: data-parallel over batch: B=256 -> 8 cores x 32.

Per-core algorithm (B_loc=32, exact fp32, bit-identical to the jax ref):
  forward t=1..T-1:
    s_t[b,j] = max_i(s_{t-1}[b,i] + trans[i,j]) + em[b,t,j]
  layout: partitions p=(ic*32+b), ic=0..3 reduce i-subrange [64ic,64ic+64)
    64 fused scalar_tensor_tensor ops: acc = max(acc, trans_rep_k + s_col_k)
    combine: copy groups 1..3 down + 3 chained TT max -> m32 rows [32,256],
    fold [32,256]->[128,64], add folded emissions, store folded lattice.
  backtrack (recompute argmax instead of storing backpointers):
    tag_T-1 = argmax_j s_T-1[b,j]
    tag_t = argmax_i(s_t[b,i] + trans[i, tag_{t+1}])   (first-index ties)
    trans^T row gather via gpsimd indirect DMA fused with +s_t (compute_op=add);
    two independent batch-half chains interleaved to hide serial latency.
"""

import numpy as np

B, T, K = 256, 512, 256
NCORES = 8
BLOC = B // NCORES  # 32
OUT_T = T + 2  # 514
BIGN = float(T)  # iota shift: iota_neg = i - 512 (negative for all i < 512)
NCHAIN = 2  # backtrack chains (batch split)


def build_program(t_steps: int = T):
    """Build the SPMD Bass program (same program for all 8 cores)."""
    from contextlib import ExitStack

    import concourse.bass as bass
    import concourse.tile as tile
    from concourse import bacc, mybir

    FP32 = mybir.dt.float32
    INT32 = mybir.dt.int32
    A = mybir.AluOpType

    nc = bacc.Bacc("TRN2", target_bir_lowering=False, num_devices=NCORES)

    # ---- DRAM I/O ----
    # em_f[t, ic*32+b, k] = emissions[b, t, 64*ic+k]  (folded layout)
    em_f_d = nc.dram_tensor("em_f", [t_steps, 128, 64], FP32, kind="ExternalInput")
    trans_rep_d = nc.dram_tensor("trans_rep", [64, 128, K], FP32, kind="ExternalInput")
    transT_d = nc.dram_tensor("transT", [K, K], FP32, kind="ExternalInput")
    iota_neg_d = nc.dram_tensor("iota_neg", [BLOC, K], FP32, kind="ExternalInput")
    eye32_d = nc.dram_tensor("eye32", [BLOC, BLOC], FP32, kind="ExternalInput")
    eye128_d = nc.dram_tensor("eye128", [128, 128], FP32, kind="ExternalInput")
    tags_d = nc.dram_tensor("tags", [BLOC, OUT_T], INT32, kind="ExternalOutput")
    # folded lattice: lat[t, ic*32+b, k] = s_t[b, 64*ic+k]
    lat_d = nc.dram_tensor("lat", [t_steps, 128, 64], FP32)

    with tile.TileContext(nc) as tc:
        with ExitStack() as ctx:
            static_pool = ctx.enter_context(tc.tile_pool(name="static", bufs=1))
            state_pool = ctx.enter_context(tc.tile_pool(name="state", bufs=3))
            pre_pool = ctx.enter_context(tc.tile_pool(name="pre", bufs=2, space="PSUM"))
            acc_pool = ctx.enter_context(tc.tile_pool(name="acc", bufs=2))
            acc2_pool = ctx.enter_context(tc.tile_pool(name="acc2", bufs=2))
            em_pool = ctx.enter_context(tc.tile_pool(name="em", bufs=6))
            h_pool = ctx.enter_context(tc.tile_pool(name="h", bufs=2))
            row_pool = ctx.enter_context(tc.tile_pool(name="row", bufs=3))
            bt_pool = ctx.enter_context(tc.tile_pool(name="bt", bufs=12))
            sm_pool = ctx.enter_context(tc.tile_pool(name="sm", bufs=4))

            # ---- static loads ----
            trans_rep = static_pool.tile([128, 64, K], FP32)
            nc.sync.dma_start(trans_rep[:], trans_rep_d.ap().transpose([1, 0, 2]))
            iota_neg = static_pool.tile([BLOC, K], FP32)
            nc.sync.dma_start(iota_neg[:], iota_neg_d.ap())
            eye32 = static_pool.tile([BLOC, BLOC], FP32)
            nc.sync.dma_start(eye32[:], eye32_d.ap())
            eye128 = static_pool.tile([128, 128], FP32)
            nc.sync.dma_start(eye128[:], eye128_d.ap())
            CHB = [(BLOC * c // NCHAIN, BLOC * (c + 1) // NCHAIN) for c in range(NCHAIN)]
            tags_fc = [
                static_pool.tile([hi - lo, T], FP32, name=f"tagsf{c}", tag=f"tagsf{c}")
                for c, (lo, hi) in enumerate(CHB)
            ]

            em_tiles = {}

            def em_load(t):
                if t >= t_steps:
                    return
                em_t = em_pool.tile([128, 64], FP32)
                nc.scalar.dma_start(em_t[:], em_f_d.ap()[t])
                em_tiles[t] = em_t

            # ---- t = 0: s_0 = em_0 ----
            s = state_pool.tile([128, 64], FP32)
            nc.sync.dma_start(s[:], em_f_d.ap()[0])
            nc.scalar.dma_start(lat_d.ap()[0], em_f_d.ap()[0])
            for t in (1, 2, 3):
                em_load(t)

            # ---- forward scan ----
            for t in range(1, t_steps):
                # two alternating accumulators: consecutive DVE stt ops are
                # independent, hiding any RMW turnaround bubble
                acc = acc_pool.tile([128, K], FP32)
                acc2 = acc2_pool.tile([128, K], FP32)
                nc.vector.tensor_scalar(
                    acc[:], trans_rep[:, 0, :], s[:, 0:1], None, op0=A.add
                )
                nc.vector.tensor_scalar(
                    acc2[:], trans_rep[:, 1, :], s[:, 1:2], None, op0=A.add
                )
                em_load(t + 3)  # issued early; executes during the stt block
                for k in range(2, 64):
                    a = acc if k % 2 == 0 else acc2
                    nc.vector.scalar_tensor_tensor(
                        a[:], trans_rep[:, k, :], s[:, k : k + 1], a[:],
                        op0=A.add, op1=A.max,
                    )
                nc.vector.tensor_tensor(out=acc[:], in0=acc[:], in1=acc2[:], op=A.max)
                # combine 4 partition-group partials: copy groups 1..3 down to
                # partitions 0-31, 2-level max tree (t2 doesn't wait on t1)
                g1 = h_pool.tile([BLOC, K], FP32, tag="g1")
                g2 = h_pool.tile([BLOC, K], FP32, tag="g2")
                g3 = h_pool.tile([BLOC, K], FP32, tag="g3")
                nc.sync.dma_start(g1[:], acc[BLOC : 2 * BLOC, :])
                nc.scalar.dma_start(g2[:], acc[2 * BLOC : 3 * BLOC, :])
                nc.sync.dma_start(g3[:], acc[3 * BLOC : 4 * BLOC, :])
                t1 = h_pool.tile([BLOC, K], FP32, tag="t1")
                t2 = h_pool.tile([BLOC, K], FP32, tag="t2")
                nc.vector.tensor_tensor(out=t1[:], in0=acc[0:BLOC, :], in1=g1[:], op=A.max)
                nc.vector.tensor_tensor(out=t2[:], in0=g2[:], in1=g3[:], op=A.max)
                m32 = row_pool.tile([BLOC, K], FP32)
                nc.vector.tensor_tensor(out=m32[:], in0=t1[:], in1=t2[:], op=A.max)
                # fold max-only rows via PE: 4 tiny matmuls with an identity
                # stationary copy m32 col-blocks to partition blocks of a PSUM
                # tile, then one DVE add folds in the emissions
                pre = pre_pool.tile([128, 64], FP32)
                em_t = em_tiles.pop(t)
                for ic in range(4):
                    nc.tensor.matmul(
                        pre[ic * BLOC : (ic + 1) * BLOC, :],
                        eye32[:],
                        m32[:, ic * 64 : ic * 64 + 64],
                        start=True,
                        stop=False,
                        tile_position=(0, ic * BLOC),
                        skip_group_check=True,
                    )
                # 5th matmul accumulates the emissions into the same PSUM
                # region; the chain reads its scalars straight from PSUM
                nc.tensor.matmul(
                    pre[:],
                    eye128[:],
                    em_t[:],
                    start=False,
                    stop=True,
                    tile_position=(0, 0),
                    skip_group_check=True,
                )
                s = pre
                # SBUF copy only for the DRAM lat store (off critical path)
                s_sb = state_pool.tile([128, 64], FP32)
                nc.vector.tensor_copy(out=s_sb[:], in_=pre[:])
                nc.scalar.dma_start(lat_d.ap()[t], s_sb[:])

            # ---- backtrack: NCHAIN independent batch-slice chains ----
            def lat_rows(t, lo, hi):
                # [hi-lo, K] row view of folded lat[t]:
                # dst[b, 64*ic+k] = lat[t, ic*32 + lo + b, k]
                return lat_d.ap()[t].rearrange("(ic bb) k -> bb ic k", ic=4)[lo:hi]

            def argmax_step(val, t_col, c):
                # top-8 values then first-occurrence index match: argmax with
                # first-index tie semantics in 2 DVE ops, index out as uint32
                nb = CHB[c][1] - CHB[c][0]
                m8 = sm_pool.tile([nb, 8], FP32, name=f"am{c}", tag=f"m{c}")
                nc.vector.max(m8[:], val[:])
                idx8 = sm_pool.tile(
                    [nb, 8], mybir.dt.uint32, name=f"aidx{c}", tag=f"idx{c}"
                )
                nc.vector.max_index(idx8[:], m8[:], val[:])
                nc.scalar.copy(tags_fc[c][:, t_col : t_col + 1], idx8[:, 0:1])
                return idx8

            idxs = [None] * NCHAIN
            for c, (lo, hi) in enumerate(CHB):
                sv = bt_pool.tile([hi - lo, K], FP32, name=f"sv{c}", tag=f"sv{c}")
                nc.sync.dma_start(sv[:], lat_rows(t_steps - 1, lo, hi))
                idxs[c] = argmax_step(sv, t_steps - 1, c)

            for t in range(t_steps - 2, -1, -1):
                svs = []
                for c, (lo, hi) in enumerate(CHB):
                    sv = bt_pool.tile([hi - lo, K], FP32, name=f"svl{c}", tag=f"sv{c}")
                    eng = nc.sync if c % 2 == 0 else nc.scalar
                    eng.dma_start(sv[:], lat_rows(t, lo, hi))
                    nc.gpsimd.indirect_dma_start(
                        out=sv[:],
                        out_offset=None,
                        in_=transT_d.ap(),
                        in_offset=bass.IndirectOffsetOnAxis(ap=idxs[c][:, :1], axis=0),
                        compute_op=A.add,
                    )
                    svs.append(sv)
                for c in range(NCHAIN):
                    idxs[c] = argmax_step(svs[c], t, c)

            # ---- output assembly (per chain; DMA merges partition offsets) ----
            for c, (lo, hi) in enumerate(CHB):
                tags_i = static_pool.tile(
                    [hi - lo, OUT_T], INT32, name=f"tagsi{c}", tag=f"tagsi{c}"
                )
                nc.vector.memset(tags_i[:], 0)
                nc.vector.tensor_scalar(
                    tags_i[:, 0:t_steps], tags_fc[c][:, 0:t_steps], 0.0, None, op0=A.add
                )
                nc.sync.dma_start(tags_d.ap()[lo:hi, :], tags_i[:])

    nc.compile()
    return nc


def _prep_inputs(emissions, transitions, t_steps: int = T):
    """Host-side layout prep. Returns per-core list of input dicts."""
    emissions = np.ascontiguousarray(emissions[:, :t_steps, :], dtype=np.float32)
    transitions = np.ascontiguousarray(transitions, dtype=np.float32)

    # trans_rep[k, ic*32+b, j] = trans[64*ic+k, j]
    tr = transitions.reshape(4, 64, K).transpose(1, 0, 2)  # [64, 4, K]
    trans_rep = np.broadcast_to(tr[:, :, None, :], (64, 4, BLOC, K)).reshape(64, 128, K)
    trans_rep = np.ascontiguousarray(trans_rep)
    transT = np.ascontiguousarray(transitions.T)
    iota_neg = np.ascontiguousarray(
        np.broadcast_to((np.arange(K, dtype=np.float32) - BIGN)[None, :], (BLOC, K))
    )
    eye32 = np.eye(BLOC, dtype=np.float32)
    eye128 = np.eye(128, dtype=np.float32)

    in_maps = []
    for c in range(NCORES):
        em_c = emissions[c * BLOC : (c + 1) * BLOC]  # [32, t, K]
        # em_f[t, ic*32+b, k] = em_c[b, t, 64*ic+k]
        em_f = np.ascontiguousarray(
            em_c.reshape(BLOC, t_steps, 4, 64)
            .transpose(1, 2, 0, 3)
            .reshape(t_steps, 128, 64)
        )
        in_maps.append(
            {
                "em_f": em_f,
                "trans_rep": trans_rep,
                "transT": transT,
                "iota_neg": iota_neg,
                "eye32": eye32,
                "eye128": eye128,
            }
        )
    return in_maps


def kernel(emissions, transitions, mask, max_sequence_length):
    from concourse.bass_utils import run_bass_kernel_spmd

    emissions = np.asarray(emissions)
    transitions = np.asarray(transitions)
    mask = np.asarray(mask)

    nc = build_program(T)
    in_maps = _prep_inputs(emissions, transitions, T)
    res = run_bass_kernel_spmd(nc, in_maps, list(range(NCORES)))
    tags = np.concatenate([res.results[c]["tags"] for c in range(NCORES)], axis=0)
    tags = tags.astype(np.int32)
    tags[:, :T] *= mask.astype(np.int32)
    return tags



# revision 4
# speedup vs baseline: 1.5448x; 1.5448x over previous
"""CRF Viterbi decode (nn_CrfDecodeLayer) Trainium2 Bass kernel.

Problem: B=256, T=512, K=256 tags. Forward max-plus scan over T with
transition matrix trans[K,K], then backtrack to recover argmax tag path.
Output: tags [B, 514] int32 (padded to max_sequence_length + 2).

Sharding: data-parallel over batch: B=256 -> 8 cores x 32.

Per-core algorithm (B_loc=32, exact fp32, bit-identical to the jax ref):
  forward t=1..T-1 via ONE custom DVE op (MAXPLUS_ACC_ANT):
    body = Src0 + Src1 + C0, accum = MAX:
      accum_out[p] = max_i(tr[i, j(p,m)] + s[b(p), i] + em[b(p), t, j(p,m)])
    64 instrs/step, partition p=(jq*32+b), instr m covers j = 64*jq + m.
    accum_out written into SNEW [128, 64] = the folded lattice layout
    lat[t][(jq,b), m] = s_t[b, 64*jq+m] (em included; exact: adding a
    per-j constant before an exact max-fold == adding it after).
  state unfold: 4 PE matmuls (stationary rep32 = eye32 tiled 4x) move
    SNEW group-rows to all partitions -> pre PSUM [128,256] = s_t[b, i]
    replicated over the 4 partition groups; one DVE copy -> SBUF s_rep
    (the Src1 stream of step t+1).
  backtrack (recompute argmax instead of storing backpointers):
    tag_T-1 = argmax_j s_T-1[b,j]
    tag_t = argmax_i(s_t[b,i] + trans[i, tag_{t+1}])   (first-index ties)
    trans^T row gather via gpsimd indirect DMA fused with +s_t
    (compute_op=add); two batch-half chains interleaved.
"""

import numpy as np

B, T, K = 256, 512, 256
NCORES = 8
BLOC = B // NCORES  # 32
OUT_T = T + 2  # 514
NCHAIN = 2  # backtrack chains (batch split)

_MAXPLUS = None


def _register_maxplus():
    """Register the custom DVE op (idempotent): accum_out = max-fold of
    (Src0 + Src1 + C0)."""
    global _MAXPLUS
    if _MAXPLUS is not None:
        return _MAXPLUS
    import concourse.dve_ops as dve_ops
    from concourse.dve_spec import Spec, Src0, Src1, C0, maxx, lower, _has_src1
    from concourse.dve_uop import DveOpSpec

    name = "MAXPLUS_ACC_ANT"
    for o in dve_ops.OPS:
        if o.name == name:
            _MAXPLUS = o
            return o
    spec = Spec(body=Src0 + Src1 + C0, accum=maxx)
    row = dve_ops._CUSTOM_DVE_ROW_BASE + len(dve_ops.OPS)
    assert row < 0x20
    shas = {}
    for ver in ("v3", "v4"):
        s = DveOpSpec(name=name, opcode=row, uops=lower(spec, ver=ver),
                      rd1_en=_has_src1(spec))
        shas[ver] = s.sha(ver)
    op = dve_ops.DveOp(name, spec, subdim=False, uops_sha=shas)
    dve_ops.OPS.append(op)
    dve_ops._SUB_OPCODE_FOR_NAME[name] = row
    _MAXPLUS = op
    return op


def build_program(t_steps: int = T):
    """Build the SPMD Bass program (same program for all 8 cores)."""
    from contextlib import ExitStack

    import concourse.bass as bass
    import concourse.tile as tile
    from concourse import bacc, mybir

    MAXPLUS = _register_maxplus()

    FP32 = mybir.dt.float32
    INT32 = mybir.dt.int32
    A = mybir.AluOpType

    nc = bacc.Bacc("TRN2", target_bir_lowering=False, num_devices=NCORES)

    # ---- DRAM I/O ----
    # em_f[t, jq*32+b, m] = emissions[b, t, 64*jq+m]  (folded layout)
    em_f_d = nc.dram_tensor("em_f", [t_steps, 128, 64], FP32, kind="ExternalInput")
    # trs[m, jq*32+b, i] = trans[i, 64*jq+m]  (stream layout, b-replicated)
    trs_d = nc.dram_tensor("trs", [64, 128, K], FP32, kind="ExternalInput")
    transT_d = nc.dram_tensor("transT", [K, K], FP32, kind="ExternalInput")
    sel4_d = nc.dram_tensor("sel4", [4, 128, 128], FP32, kind="ExternalInput")
    tags_d = nc.dram_tensor("tags", [BLOC, OUT_T], INT32, kind="ExternalOutput")
    # folded lattice: lat[t, jq*32+b, m] = s_t[b, 64*jq+m]
    lat_d = nc.dram_tensor("lat", [t_steps, 128, 64], FP32)

    with tile.TileContext(nc) as tc:
        with ExitStack() as ctx:
            static_pool = ctx.enter_context(tc.tile_pool(name="static", bufs=1))
            srep_pool = ctx.enter_context(tc.tile_pool(name="srep", bufs=2))
            pre_pool = ctx.enter_context(tc.tile_pool(name="pre", bufs=2, space="PSUM"))
            snew_pool = ctx.enter_context(tc.tile_pool(name="snew", bufs=3))
            scr_pool = ctx.enter_context(tc.tile_pool(name="scr", bufs=2))
            em_pool = ctx.enter_context(tc.tile_pool(name="em", bufs=6))
            bt_pool = ctx.enter_context(tc.tile_pool(name="bt", bufs=12))
            sm_pool = ctx.enter_context(tc.tile_pool(name="sm", bufs=4))

            # ---- static loads ----
            trs = static_pool.tile([128, 64, K], FP32)
            nc.sync.dma_start(trs[:], trs_d.ap().transpose([1, 0, 2]))
            sel4 = static_pool.tile([128, 4, 128], FP32)
            nc.sync.dma_start(sel4[:], sel4_d.ap().transpose([1, 0, 2]))
            CHB = [(BLOC * c // NCHAIN, BLOC * (c + 1) // NCHAIN) for c in range(NCHAIN)]
            tags_fc = [
                static_pool.tile([hi - lo, T], FP32, name=f"tagsf{c}", tag=f"tagsf{c}")
                for c, (lo, hi) in enumerate(CHB)
            ]

            em_tiles = {}

            def em_load(t):
                if t >= t_steps:
                    return
                em_t = em_pool.tile([128, 64], FP32)
                nc.scalar.dma_start(em_t[:], em_f_d.ap()[t])
                em_tiles[t] = em_t

            def unfold(snew_like):
                """4 PE matmuls: pre[:, 64*jq : 64*jq+64] = rep32^T-style
                replication of snew rows (jq*32 .. jq*32+32); then DVE copy
                to SBUF s_rep [128, 256] (the next Src1 stream)."""
                pre = pre_pool.tile([128, K], FP32)
                for jq in range(4):
                    nc.tensor.matmul(
                        pre[:, jq * 64 : jq * 64 + 64],
                        sel4[:, jq, :],
                        snew_like[:],
                        start=True,
                        stop=True,
                    )
                s_rep = srep_pool.tile([128, K], FP32)
                nc.vector.tensor_copy(out=s_rep[:], in_=pre[:])
                return s_rep

            # ---- t = 0: s_0 = em_0 ----
            em0 = snew_pool.tile([128, 64], FP32)
            nc.sync.dma_start(em0[:], em_f_d.ap()[0])
            nc.gpsimd.dma_start(lat_d.ap()[0], em_f_d.ap()[0])
            s_rep = unfold(em0)
            for t in (1, 2, 3):
                em_load(t)

            # ---- forward scan ----
            for t in range(1, t_steps):
                em_t = em_tiles.pop(t)
                snew = snew_pool.tile([128, 64], FP32)
                em_load(t + 3)
                for m in range(64):
                    scr = scr_pool.tile([128, K], FP32, name=f"scr{m % 2}", tag=f"scr{m % 2}")
                    nc.vector._custom_dve(
                        MAXPLUS,
                        out=scr[:],
                        in0=trs[:, m, :],
                        in1=s_rep[:],
                        s0=em_t[:, m : m + 1],
                        accum_out=snew[:, m : m + 1],
                    )
                nc.gpsimd.dma_start(lat_d.ap()[t], snew[:])
                if t < t_steps - 1:
                    s_rep = unfold(snew)

            # ---- backtrack: NCHAIN independent batch-slice chains ----
            def lat_rows(t, lo, hi):
                # [hi-lo, K] row view of folded lat[t]:
                # dst[b, 64*jq+m] = lat[t, jq*32 + lo + b, m]
                return lat_d.ap()[t].rearrange("(jq bb) m -> bb jq m", jq=4)[lo:hi]

            def argmax_step(val, t_col, c):
                # top-8 values then first-occurrence index match: argmax with
                # first-index tie semantics in 2 DVE ops, index out as uint32
                nb = CHB[c][1] - CHB[c][0]
                m8 = sm_pool.tile([nb, 8], FP32, name=f"am{c}", tag=f"m{c}")
                nc.vector.max(m8[:], val[:])
                idx8 = sm_pool.tile(
                    [nb, 8], mybir.dt.uint32, name=f"aidx{c}", tag=f"idx{c}"
                )
                nc.vector.max_index(idx8[:], m8[:], val[:])
                nc.scalar.copy(tags_fc[c][:, t_col : t_col + 1], idx8[:, 0:1])
                return idx8

            idxs = [None] * NCHAIN
            for c, (lo, hi) in enumerate(CHB):
                sv = bt_pool.tile([hi - lo, K], FP32, name=f"sv{c}", tag=f"sv{c}")
                nc.sync.dma_start(sv[:], lat_rows(t_steps - 1, lo, hi))
                idxs[c] = argmax_step(sv, t_steps - 1, c)

            for t in range(t_steps - 2, -1, -1):
                svs = []
                for c, (lo, hi) in enumerate(CHB):
                    sv = bt_pool.tile([hi - lo, K], FP32, name=f"svl{c}", tag=f"sv{c}")
                    eng = nc.sync if c % 2 == 0 else nc.scalar
                    eng.dma_start(sv[:], lat_rows(t, lo, hi))
                    nc.gpsimd.indirect_dma_start(
                        out=sv[:],
                        out_offset=None,
                        in_=transT_d.ap(),
                        in_offset=bass.IndirectOffsetOnAxis(ap=idxs[c][:, :1], axis=0),
                        compute_op=A.add,
                    )
                    svs.append(sv)
                for c in range(NCHAIN):
                    idxs[c] = argmax_step(svs[c], t, c)

            # ---- output assembly (per chain; DMA merges partition offsets) ----
            for c, (lo, hi) in enumerate(CHB):
                tags_i = static_pool.tile(
                    [hi - lo, OUT_T], INT32, name=f"tagsi{c}", tag=f"tagsi{c}"
                )
                nc.vector.memset(tags_i[:], 0)
                nc.vector.tensor_scalar(
                    tags_i[:, 0:t_steps], tags_fc[c][:, 0:t_steps], 0.0, None, op0=A.add
                )
                nc.sync.dma_start(tags_d.ap()[lo:hi, :], tags_i[:])

    nc.compile()
    return nc


def _prep_inputs(emissions, transitions, t_steps: int = T):
    """Host-side layout prep. Returns per-core list of input dicts."""
    emissions = np.ascontiguousarray(emissions[:, :t_steps, :], dtype=np.float32)
    transitions = np.ascontiguousarray(transitions, dtype=np.float32)

    # trs[m, jq*32+b, i] = trans[i, 64*jq+m] = transT[64*jq+m, i]
    trT = transitions.T.reshape(4, 64, K).transpose(1, 0, 2)  # [64 m, 4 jq, K i]
    trs = np.broadcast_to(trT[:, :, None, :], (64, 4, BLOC, K)).reshape(64, 128, K)
    trs = np.ascontiguousarray(trs)
    transT = np.ascontiguousarray(transitions.T)
    # sel4[jq][k, p] = 1 iff k == jq*32 + (p % 32) (unfold stationaries)
    kk = np.arange(128)[:, None]
    pp = np.arange(128)[None, :]
    sel4 = np.stack([(kk == jq * BLOC + (pp % BLOC)).astype(np.float32)
                     for jq in range(4)])
    sel4 = np.ascontiguousarray(sel4)

    in_maps = []
    for c in range(NCORES):
        em_c = emissions[c * BLOC : (c + 1) * BLOC]  # [32, t, K]
        # em_f[t, jq*32+b, m] = em_c[b, t, 64*jq+m]
        em_f = np.ascontiguousarray(
            em_c.reshape(BLOC, t_steps, 4, 64)
            .transpose(1, 2, 0, 3)
            .reshape(t_steps, 128, 64)
        )
        in_maps.append(
            {
                "em_f": em_f,
                "trs": trs,
                "transT": transT,
                "sel4": sel4,
            }
        )
    return in_maps


def kernel(emissions, transitions, mask, max_sequence_length):
    from concourse.bass_utils import run_bass_kernel_spmd

    emissions = np.asarray(emissions)
    transitions = np.asarray(transitions)
    mask = np.asarray(mask)

    nc = build_program(T)
    in_maps = _prep_inputs(emissions, transitions, T)
    res = run_bass_kernel_spmd(nc, in_maps, list(range(NCORES)))
    tags = np.concatenate([res.results[c]["tags"] for c in range(NCORES)], axis=0)
    tags = tags.astype(np.int32)
    tags[:, :T] *= mask.astype(np.int32)
    return tags


# revision 8
# speedup vs baseline: 1.5510x; 1.0040x over previous
"""CRF Viterbi decode (nn_CrfDecodeLayer) Trainium2 Bass kernel.

Problem: B=256, T=512, K=256 tags. Forward max-plus scan over T with
transition matrix trans[K,K], then backtrack to recover argmax tag path.
Output: tags [B, 514] int32 (padded to max_sequence_length + 2).

Sharding: data-parallel over batch: B=256 -> 8 cores x 32.

Per-core algorithm (B_loc=32, exact fp32, bit-identical to the jax ref):
  forward t=1..T-1 via ONE custom DVE op (MAXPLUS_ACC_ANT):
    body = Src0 + Src1 + C0, accum = MAX:
      accum_out[p] = max_i(tr[i, j(p,m)] + s[b(p), i] + em[b(p), t, j(p,m)])
    64 instrs/step, partition p=(jq*32+b), instr m covers j = 64*jq + m.
    accum_out written into SNEW [128, 64] = the folded lattice layout
    lat[t][(jq,b), m] = s_t[b, 64*jq+m] (em included; exact: adding a
    per-j constant before an exact max-fold == adding it after).
  state unfold: 4 PE matmuls (stationary rep32 = eye32 tiled 4x) move
    SNEW group-rows to all partitions -> pre PSUM [128,256] = s_t[b, i]
    replicated over the 4 partition groups; one DVE copy -> SBUF s_rep
    (the Src1 stream of step t+1).
  backtrack (recompute argmax instead of storing backpointers):
    tag_T-1 = argmax_j s_T-1[b,j]
    tag_t = argmax_i(s_t[b,i] + trans[i, tag_{t+1}])   (first-index ties)
    trans^T row gather via gpsimd indirect DMA fused with +s_t
    (compute_op=add); two batch-half chains interleaved.
"""

import numpy as np

B, T, K = 256, 512, 256
NCORES = 8
BLOC = B // NCORES  # 32
OUT_T = T + 2  # 514
NCHAIN = 1  # backtrack chains (batch split)

_MAXPLUS = None


def _register_maxplus():
    """Register the custom DVE op (idempotent): accum_out = max-fold of
    (Src0 + Src1 + C0)."""
    global _MAXPLUS
    if _MAXPLUS is not None:
        return _MAXPLUS
    import concourse.dve_ops as dve_ops
    from concourse.dve_spec import Spec, Src0, Src1, C0, maxx, lower, _has_src1
    from concourse.dve_uop import DveOpSpec

    name = "MAXPLUS_ACC_ANT"
    for o in dve_ops.OPS:
        if o.name == name:
            _MAXPLUS = o
            return o
    spec = Spec(body=Src0 + Src1 + C0, accum=maxx)
    row = dve_ops._CUSTOM_DVE_ROW_BASE + len(dve_ops.OPS)
    assert row < 0x20
    shas = {}
    for ver in ("v3", "v4"):
        s = DveOpSpec(name=name, opcode=row, uops=lower(spec, ver=ver),
                      rd1_en=_has_src1(spec))
        shas[ver] = s.sha(ver)
    op = dve_ops.DveOp(name, spec, subdim=False, uops_sha=shas)
    dve_ops.OPS.append(op)
    dve_ops._SUB_OPCODE_FOR_NAME[name] = row
    _MAXPLUS = op
    return op


def build_program(t_steps: int = T):
    """Build the SPMD Bass program (same program for all 8 cores)."""
    from contextlib import ExitStack

    import concourse.bass as bass
    import concourse.tile as tile
    from concourse import bacc, mybir

    MAXPLUS = _register_maxplus()

    FP32 = mybir.dt.float32
    INT32 = mybir.dt.int32
    A = mybir.AluOpType

    nc = bacc.Bacc("TRN2", target_bir_lowering=False, num_devices=NCORES)

    # ---- DRAM I/O ----
    # em_f[t, jq*32+b, m] = emissions[b, t, 64*jq+m]  (folded layout)
    em_f_d = nc.dram_tensor("em_f", [t_steps, 128, 64], FP32, kind="ExternalInput")
    # trs[m, jq*32+b, i] = trans[i, 64*jq+m]  (stream layout, b-replicated)
    trs_d = nc.dram_tensor("trs", [64, 128, K], FP32, kind="ExternalInput")
    transT_d = nc.dram_tensor("transT", [K, K], FP32, kind="ExternalInput")
    sel4_d = nc.dram_tensor("sel4", [4, 128, 128], FP32, kind="ExternalInput")
    tags_d = nc.dram_tensor("tags", [BLOC, OUT_T], INT32, kind="ExternalOutput")
    # folded lattice: lat[t, jq*32+b, m] = s_t[b, 64*jq+m]
    lat_d = nc.dram_tensor("lat", [t_steps, 128, 64], FP32)

    with tile.TileContext(nc) as tc:
        with ExitStack() as ctx:
            static_pool = ctx.enter_context(tc.tile_pool(name="static", bufs=1))
            srep_pool = ctx.enter_context(tc.tile_pool(name="srep", bufs=2))
            pre_pool = ctx.enter_context(tc.tile_pool(name="pre", bufs=2, space="PSUM"))
            snew_pool = ctx.enter_context(tc.tile_pool(name="snew", bufs=3))
            scr_pool = ctx.enter_context(tc.tile_pool(name="scr", bufs=2))
            em_pool = ctx.enter_context(tc.tile_pool(name="em", bufs=6))
            bt_pool = ctx.enter_context(tc.tile_pool(name="bt", bufs=12))
            sm_pool = ctx.enter_context(tc.tile_pool(name="sm", bufs=4))

            # ---- static loads ----
            trs = static_pool.tile([128, 64, K], FP32)
            nc.sync.dma_start(trs[:], trs_d.ap().transpose([1, 0, 2]))
            sel4 = static_pool.tile([128, 4, 128], FP32)
            nc.sync.dma_start(sel4[:], sel4_d.ap().transpose([1, 0, 2]))
            CHB = [(BLOC * c // NCHAIN, BLOC * (c + 1) // NCHAIN) for c in range(NCHAIN)]
            tags_fc = [
                static_pool.tile([hi - lo, T], FP32, name=f"tagsf{c}", tag=f"tagsf{c}")
                for c, (lo, hi) in enumerate(CHB)
            ]

            em_tiles = {}

            def em_load(t):
                if t >= t_steps:
                    return
                em_t = em_pool.tile([128, 64], FP32)
                nc.scalar.dma_start(em_t[:], em_f_d.ap()[t])
                em_tiles[t] = em_t

            def unfold_mm(pre, snew_like, mlo, mhi):
                """pre[:, 64*jq+m] (m in [mlo,mhi)) = snew[jq*32 + p%32, m]
                via 4 PE matmuls. Depends only on snew columns [mlo, mhi)."""
                for jq in range(4):
                    nc.tensor.matmul(
                        pre[:, jq * 64 + mlo : jq * 64 + mhi],
                        sel4[:, jq, :],
                        snew_like[:, mlo:mhi],
                        start=True,
                        stop=True,
                    )

            def unfold_copy(pre, s_rep, mlo, mhi):
                pv = pre[:].rearrange("p (jq m) -> p jq m", jq=4)[:, :, mlo:mhi]
                sv = s_rep[:].rearrange("p (jq m) -> p jq m", jq=4)[:, :, mlo:mhi]
                nc.vector.tensor_copy(out=sv, in_=pv)

            def unfold(snew_like):
                pre = pre_pool.tile([128, K], FP32)
                s_rep = srep_pool.tile([128, K], FP32)
                unfold_mm(pre, snew_like, 0, 64)
                unfold_copy(pre, s_rep, 0, 64)
                return s_rep

            # ---- t = 0: s_0 = em_0 ----
            em0 = snew_pool.tile([128, 64], FP32)
            nc.sync.dma_start(em0[:], em_f_d.ap()[0])
            nc.gpsimd.dma_start(lat_d.ap()[0], em_f_d.ap()[0])
            s_rep = unfold(em0)
            for t in (1, 2, 3):
                em_load(t)

            # ---- forward scan ----
            for t in range(1, t_steps):
                em_t = em_tiles.pop(t)
                snew = snew_pool.tile([128, 64], FP32)
                last = t == t_steps - 1
                if not last:
                    pre = pre_pool.tile([128, K], FP32)
                    s_rep_next = srep_pool.tile([128, K], FP32)
                em_load(t + 3)
                for m in range(64):
                    scr = scr_pool.tile([128, K], FP32, name=f"scr{m % 2}", tag=f"scr{m % 2}")
                    nc.vector._custom_dve(
                        MAXPLUS,
                        out=scr[:],
                        in0=trs[:, m, :],
                        in1=s_rep[:],
                        s0=em_t[:, m : m + 1],
                        accum_out=snew[:, m : m + 1],
                    )
                    if not last:
                        # first-half unfold overlaps the second half of the
                        # MAXPLUS block (PE after instr 31, DVE copy at 48
                        # when the matmuls are long done)
                        if m == 31:
                            unfold_mm(pre, snew, 0, 32)
                        elif m == 47:
                            unfold_copy(pre, s_rep_next, 0, 32)
                nc.gpsimd.dma_start(lat_d.ap()[t], snew[:])
                if not last:
                    unfold_mm(pre, snew, 32, 64)
                    unfold_copy(pre, s_rep_next, 32, 64)
                    s_rep = s_rep_next

            # ---- backtrack: NCHAIN independent batch-slice chains ----
            def lat_rows(t, lo, hi):
                # [hi-lo, K] row view of folded lat[t]:
                # dst[b, 64*jq+m] = lat[t, jq*32 + lo + b, m]
                return lat_d.ap()[t].rearrange("(jq bb) m -> bb jq m", jq=4)[lo:hi]

            def argmax_step(val, t_col, c):
                # top-8 values then first-occurrence index match: argmax with
                # first-index tie semantics in 2 DVE ops, index out as uint32
                nb = CHB[c][1] - CHB[c][0]
                m8 = sm_pool.tile([nb, 8], FP32, name=f"am{c}", tag=f"m{c}")
                nc.vector.max(m8[:], val[:])
                idx8 = sm_pool.tile(
                    [nb, 8], mybir.dt.uint32, name=f"aidx{c}", tag=f"idx{c}"
                )
                nc.vector.max_index(idx8[:], m8[:], val[:])
                nc.scalar.copy(tags_fc[c][:, t_col : t_col + 1], idx8[:, 0:1])
                return idx8

            idxs = [None] * NCHAIN
            for c, (lo, hi) in enumerate(CHB):
                sv = bt_pool.tile([hi - lo, K], FP32, name=f"sv{c}", tag=f"sv{c}")
                nc.sync.dma_start(sv[:], lat_rows(t_steps - 1, lo, hi))
                idxs[c] = argmax_step(sv, t_steps - 1, c)

            for t in range(t_steps - 2, -1, -1):
                svs = []
                for c, (lo, hi) in enumerate(CHB):
                    sv = bt_pool.tile([hi - lo, K], FP32, name=f"svl{c}", tag=f"sv{c}")
                    eng = nc.sync if c % 2 == 0 else nc.scalar
                    eng.dma_start(sv[:], lat_rows(t, lo, hi))
                    nc.gpsimd.indirect_dma_start(
                        out=sv[:],
                        out_offset=None,
                        in_=transT_d.ap(),
                        in_offset=bass.IndirectOffsetOnAxis(ap=idxs[c][:, :1], axis=0),
                        compute_op=A.add,
                    )
                    svs.append(sv)
                for c in range(NCHAIN):
                    idxs[c] = argmax_step(svs[c], t, c)

            # ---- output assembly (per chain; DMA merges partition offsets) ----
            for c, (lo, hi) in enumerate(CHB):
                tags_i = static_pool.tile(
                    [hi - lo, OUT_T], INT32, name=f"tagsi{c}", tag=f"tagsi{c}"
                )
                nc.vector.memset(tags_i[:], 0)
                nc.vector.tensor_scalar(
                    tags_i[:, 0:t_steps], tags_fc[c][:, 0:t_steps], 0.0, None, op0=A.add
                )
                nc.sync.dma_start(tags_d.ap()[lo:hi, :], tags_i[:])

    nc.compile()
    return nc


def _prep_inputs(emissions, transitions, t_steps: int = T):
    """Host-side layout prep. Returns per-core list of input dicts."""
    emissions = np.ascontiguousarray(emissions[:, :t_steps, :], dtype=np.float32)
    transitions = np.ascontiguousarray(transitions, dtype=np.float32)

    # trs[m, jq*32+b, i] = trans[i, 64*jq+m] = transT[64*jq+m, i]
    trT = transitions.T.reshape(4, 64, K).transpose(1, 0, 2)  # [64 m, 4 jq, K i]
    trs = np.broadcast_to(trT[:, :, None, :], (64, 4, BLOC, K)).reshape(64, 128, K)
    trs = np.ascontiguousarray(trs)
    transT = np.ascontiguousarray(transitions.T)
    # sel4[jq][k, p] = 1 iff k == jq*32 + (p % 32) (unfold stationaries)
    kk = np.arange(128)[:, None]
    pp = np.arange(128)[None, :]
    sel4 = np.stack([(kk == jq * BLOC + (pp % BLOC)).astype(np.float32)
                     for jq in range(4)])
    sel4 = np.ascontiguousarray(sel4)

    in_maps = []
    for c in range(NCORES):
        em_c = emissions[c * BLOC : (c + 1) * BLOC]  # [32, t, K]
        # em_f[t, jq*32+b, m] = em_c[b, t, 64*jq+m]
        em_f = np.ascontiguousarray(
            em_c.reshape(BLOC, t_steps, 4, 64)
            .transpose(1, 2, 0, 3)
            .reshape(t_steps, 128, 64)
        )
        in_maps.append(
            {
                "em_f": em_f,
                "trs": trs,
                "transT": transT,
                "sel4": sel4,
            }
        )
    return in_maps


def kernel(emissions, transitions, mask, max_sequence_length):
    from concourse.bass_utils import run_bass_kernel_spmd

    emissions = np.asarray(emissions)
    transitions = np.asarray(transitions)
    mask = np.asarray(mask)

    nc = build_program(T)
    in_maps = _prep_inputs(emissions, transitions, T)
    res = run_bass_kernel_spmd(nc, in_maps, list(range(NCORES)))
    tags = np.concatenate([res.results[c]["tags"] for c in range(NCORES)], axis=0)
    tags = tags.astype(np.int32)
    tags[:, :T] *= mask.astype(np.int32)
    return tags
